# revision 1
# baseline (speedup 1.0000x reference)
"""Trainium2 Bass kernel for CombinedRepeatCausalLinear (parallel forward).

Computes out[b,e,t] = sum_s x[b,e,s] * W[s,t] + bias[t] where
  W[s,t] = mask(t>=s) * (w0[s]*d0^(t-s) + w1[t]*d1^(t-s))
for S = 2048, x of shape (8, 1024, 2048) fp32.

Strategy (8 NeuronCores, data-parallel over batch):
  - core c gets x[c] (1024 rows); host pre-transposes to xT (2048, 1024) so
    the contraction dim lands on SBUF partitions with contiguous DMAs.
  - W is rank-2 before causal masking: each (128 s) x (512 t) chunk of W is
    generated ON-CHIP by a K=2 matmul from tiny host-precomputed factor
    vectors (per-chunk exponent offsets keep fp32 in range), then boundary
    chunks are multiplied by one of 4 precomputed 0/1 causal masks on DVE.
  - main matmul runs in float32r (full-rate fp32 mode, 1 cyc/row at N=512):
    outT[t,r] = sum_s W[s,t] * xT[s,r], accumulated over s-tiles in PSUM,
    skipping all-zero below-diagonal blocks (272 of 512 matmuls).
  - bias is fused into the PSUM->SBUF copy on the scalar engine
    (activation Identity with per-partition bias).
  - host transposes each core's outT back and stacks.
"""

import numpy as np

import concourse.bass as bass
import concourse.mybir as mybir
import concourse.tile as tile
from concourse import bacc
from concourse.bass_utils import run_bass_kernel_spmd

F32 = mybir.dt.float32
F32R = mybir.dt.float32r

B = 8
E = 1024
S = 2048
DC = 1.0
N_CORES = 8
R = (B * E) // N_CORES      # rows per core = 1024
ST = S // 128               # 16 s-tiles of 128
TB = S // 512               # 4 t-blocks of 512
RB = R // 512               # 2 r-blocks of 512

# chunk list: (si, tb) with si <= 4*tb+3  (40 chunks)
CHUNKS = [(si, tb) for tb in range(TB) for si in range(min(ST, 4 * tb + 4))]
CHUNK_IDX = {c: i for i, c in enumerate(CHUNKS)}
N_CHUNKS = len(CHUNKS)

_PROGRAM = None  # (nc, ...) cache


def _build_program(repeats=1, no_wgen=False, no_store=False, no_xload=False,
                   po_bufs=4, wc_bufs=30, osb_bufs=4, xsplit=1):
    nc = bacc.Bacc("TRN2", target_bir_lowering=False, debug=False,
                   num_devices=N_CORES)

    xT_d = nc.declare_dram_parameter("xT", [S, R], F32, isOutput=False)
    wstat_d = nc.declare_dram_parameter("wstat", [N_CHUNKS, 2, 128], F32,
                                        isOutput=False)
    wmov_d = nc.declare_dram_parameter("wmov", [N_CHUNKS, 2, 512], F32,
                                       isOutput=False)
    masks_d = nc.declare_dram_parameter("masks", [4, 128, 512], F32,
                                        isOutput=False)
    biasT_d = nc.declare_dram_parameter("biasT", [128, ST], F32,
                                        isOutput=False)
    outT_d = nc.declare_dram_parameter("outT", [S, R], F32, isOutput=True)

    with tile.TileContext(nc) as tc:
        with (
            tc.tile_pool(name="xp", bufs=1) as xp,
            tc.tile_pool(name="cst", bufs=1) as cst,
            tc.tile_pool(name="wg", bufs=6) as wg,
            tc.tile_pool(name="wc", bufs=wc_bufs) as wcp,
            tc.tile_pool(name="osb", bufs=osb_bufs) as osb,
            tc.tile_pool(name="pw", bufs=2, space="PSUM") as pwp,
            tc.tile_pool(name="po", bufs=po_bufs, space="PSUM") as pop,
        ):
            mask_sb = []
            for m in range(4):
                mt = cst.tile([128, 512], F32, tag=f"mask{m}")
                nc.gpsimd.dma_start(mt[:], masks_d[m])
                mask_sb.append(mt)
            bias_sb = cst.tile([128, ST], F32, tag="bias")
            nc.gpsimd.dma_start(bias_sb[:], biasT_d[:])

            for rep in range(repeats):
              # resident x tiles: [128 s, 1024 r] per s-tile
              xs = []
              for si in range(ST):
                t = xp.tile([128, R], F32R, tag=f"x{si}", name=f"x{si}_{rep}")
                if not no_xload:
                    for xs_i in range(xsplit):
                        w0c = (R // xsplit) * xs_i
                        w1c = (R // xsplit) * (xs_i + 1)
                        nc.sync.dma_start(
                            t[:, w0c:w1c],
                            xT_d[128 * si:128 * (si + 1), w0c:w1c]
                            .bitcast(F32R))
                xs.append(t)
              def emit_wgen(tb):
                # generate W chunks (si, tb) for t-block tb
                w_sb = []
                for si in range(min(ST, 4 * tb + 4)):
                    w = wcp.tile([128, 512], F32R, tag="wc", name=f"w{tb}_{si}")
                    if no_wgen:
                        nc.gpsimd.memset(w[:], 0.0)
                    else:
                        ci = CHUNK_IDX[(si, tb)]
                        st = wg.tile([2, 128], F32R, tag="wstat", name="st")
                        nc.gpsimd.dma_start(st[:], wstat_d[ci].bitcast(F32R))
                        mv = wg.tile([2, 512], F32R, tag="wmov", name="mv")
                        nc.gpsimd.dma_start(mv[:], wmov_d[ci].bitcast(F32R))
                        psw = pwp.tile([128, 512], F32, tag="pw", name="psw")
                        nc.tensor.matmul(psw[:], st[:], mv[:], start=True,
                                         stop=True)
                        d2 = 4 * tb - si
                        if d2 <= 0:
                            nc.vector.tensor_mul(w[:], psw[:],
                                                 mask_sb[d2 + 3][:])
                        else:
                            nc.vector.tensor_copy(w[:], psw[:])
                    w_sb.append(w)
                return w_sb

              w_by_tb = {0: emit_wgen(0), 1: emit_wgen(1)}
              for tb in range(TB):
                w_sb = w_by_tb.pop(tb)
                for tjl in range(4):
                    tj = 4 * tb + tjl
                    out_sb = osb.tile([128, R], F32, tag="osb")
                    ps = [pop.tile([128, 512], F32, tag="po", name=f"po{rb}")
                          for rb in range(RB)]
                    for si in range(tj + 1):
                        lhsT = w_sb[si][:, 128 * tjl:128 * (tjl + 1)]
                        for rb in range(RB):
                            nc.tensor.matmul(
                                ps[rb][:], lhsT,
                                xs[si][:, 512 * rb:512 * (rb + 1)],
                                start=(si == 0), stop=(si == tj),
                            )
                    for rb in range(RB):
                        nc.scalar.activation(
                            out_sb[:, 512 * rb:512 * (rb + 1)], ps[rb][:],
                            mybir.ActivationFunctionType.Identity,
                            bias=bias_sb[:, tj:tj + 1],
                        )
                    if not no_store:
                        nc.sync.dma_start(
                            outT_d[128 * tj:128 * (tj + 1), :], out_sb[:])
                if tb + 2 < TB:
                    w_by_tb[tb + 2] = emit_wgen(tb + 2)

    nc.compile()
    return nc


def _host_prep(weight, bias, decay_value):
    w0 = weight[0].astype(np.float64)
    w1 = weight[1].astype(np.float64)
    d0 = float(np.clip(np.float32(decay_value[0, 0]), 0.9, 1.0))
    d1 = float(np.clip(np.float32(decay_value[1, 0]), 0.9, 1.0))
    ii = np.arange(128, dtype=np.float64)
    jj = np.arange(512, dtype=np.float64)

    wstat = np.zeros((N_CHUNKS, 2, 128), dtype=np.float32)
    wmov = np.zeros((N_CHUNKS, 2, 512), dtype=np.float32)
    for ci, (si, tb) in enumerate(CHUNKS):
        d2 = 4 * tb - si
        # W[i,j] = w0[i]*d0^(j-i) + w1[j]*d1^(j-i), j-i = 128*d2 + jj - ii
        wstat[ci, 0] = (w0[128 * si:128 * (si + 1)] * d0 ** (-ii / DC)
                        ).astype(np.float32)
        wstat[ci, 1] = (d1 ** ((128 * d2 - ii) / DC)).astype(np.float32)
        wmov[ci, 0] = (d0 ** ((128 * d2 + jj) / DC)).astype(np.float32)
        wmov[ci, 1] = (w1[512 * tb:512 * (tb + 1)] * d1 ** (jj / DC)
                       ).astype(np.float32)

    masks = np.zeros((4, 128, 512), dtype=np.float32)
    for m in range(4):
        d2 = m - 3
        masks[m] = (128 * d2 + jj[None, :] - ii[:, None] >= 0
                    ).astype(np.float32)

    biasT = np.ascontiguousarray(
        bias.astype(np.float32).reshape(ST, 128).T)
    return wstat, wmov, masks, biasT


def kernel(x, weight, bias, decay_value, index=0, recurrent=0, **_):
    global _PROGRAM
    x = np.asarray(x, dtype=np.float32)
    weight = np.asarray(weight, dtype=np.float32)
    bias = np.asarray(bias, dtype=np.float32)
    decay_value = np.asarray(decay_value, dtype=np.float32)

    if _PROGRAM is None:
        _PROGRAM = _build_program()
    nc = _PROGRAM

    wstat, wmov, masks, biasT = _host_prep(weight, bias, decay_value)

    x2 = x.reshape(B * E, S)
    in_maps = []
    for c in range(N_CORES):
        xT_c = np.ascontiguousarray(x2[R * c:R * (c + 1), :].T)
        in_maps.append({
            "xT": xT_c, "wstat": wstat, "wmov": wmov,
            "masks": masks, "biasT": biasT,
        })

    res = run_bass_kernel_spmd(nc, in_maps, core_ids=list(range(N_CORES)))
    out = np.empty((B * E, S), dtype=np.float32)
    for c in range(N_CORES):
        out[R * c:R * (c + 1), :] = res.results[c]["outT"].T
    return out.reshape(B, E, S)



# revision 4
# speedup vs baseline: 2.0221x; 2.0221x over previous
"""Trainium2 Bass kernel for CombinedRepeatCausalLinear (parallel forward).

Computes out[b,e,t] = sum_s x[b,e,s] * W[s,t] + bias[t] where
  W[s,t] = mask(t>=s) * (w0[s]*d0^(t-s) + w1[t]*d1^(t-s))
for S = 2048, x of shape (8, 1024, 2048) fp32.

Strategy (8 NeuronCores, data-parallel over batch):
  W is rank-2 + causal, so instead of the dense (S,S) matmul we use the
  chunked linear-recurrence form with chunk L=128 (16 chunks):
    out[t, r] = within-chunk triangular part + decayed cross-chunk state.
  Per core (r = 1024 rows, bf16 I/O, fp32 PSUM accumulation):
    - contraction: per chunk c one matmul with stationary Awide_c
      [128 s x 32] (zeros except cols 2c,2c+1) accumulates all chunk
      states into ONE PSUM tile P[32, r] -- the PE itself assembles the
      partition layout, avoiding (illegal) unaligned partition copies.
    - combine: one 32x32 matmul with host-precomputed chunk-decay matrix
      T32 turns P into per-chunk start-of-chunk states H[32, r].
    - main: per chunk, within-chunk matmul (K=128, stationary = masked
      128x128 W block) + cross matmul (K=32, stationary Dex_c zero
      except rows 2c,2c+1) accumulate in the same PSUM tile; scalar
      engine fuses bias into the PSUM->SBUF copy; DMA out as bf16.
  r is split in two 512-blocks so rb1 contractions overlap rb0 mains;
  x loads are ordered rb-major to feed the pipeline.
"""

import numpy as np
import ml_dtypes

import concourse.bass as bass
import concourse.mybir as mybir
import concourse.tile as tile
from concourse import bacc
from concourse.bass_utils import run_bass_kernel_spmd

F32 = mybir.dt.float32
BF16 = mybir.dt.bfloat16
NBF16 = ml_dtypes.bfloat16

B = 8
E = 1024
S = 2048
DC = 1.0
N_CORES = 8
R = (B * E) // N_CORES      # rows per core = 1024
L = 128                     # chunk length along s/t
NCH = S // L                # 16 chunks
RB = 2                      # r blocks
RBS = R // RB               # 512

_PROGRAM = None


def _build_program():
    nc = bacc.Bacc("TRN2", target_bir_lowering=False, debug=False,
                   num_devices=N_CORES)

    xT_d = nc.declare_dram_parameter("xT", [S, R], BF16, isOutput=False)
    wloc_d = nc.declare_dram_parameter("wloc", [NCH, L, L], BF16,
                                       isOutput=False)
    awide_d = nc.declare_dram_parameter("awide", [NCH, L, 2 * NCH], BF16,
                                        isOutput=False)
    dex_d = nc.declare_dram_parameter("dex", [NCH, 2 * NCH, L], BF16,
                                      isOutput=False)
    t32_d = nc.declare_dram_parameter("t32", [2 * NCH, 2 * NCH], BF16,
                                      isOutput=False)
    biasT_d = nc.declare_dram_parameter("biasT", [L, NCH], F32,
                                        isOutput=False)
    outT_d = nc.declare_dram_parameter("outT", [S, R], BF16, isOutput=True)

    with tile.TileContext(nc) as tc:
        with (
            tc.tile_pool(name="cst", bufs=1) as cst,
            tc.tile_pool(name="xp", bufs=1) as xp,
            tc.tile_pool(name="dr", bufs=4) as dr,
            tc.tile_pool(name="osb", bufs=4) as osb,
            tc.tile_pool(name="psc", bufs=2, space="PSUM") as psc,
            tc.tile_pool(name="psh", bufs=2, space="PSUM") as psh,
            tc.tile_pool(name="pop", bufs=4, space="PSUM") as pop,
        ):
            # resident constants (small, gpsimd DMA queue)
            t32_sb = cst.tile([2 * NCH, 2 * NCH], BF16, tag="t32")
            nc.gpsimd.dma_start(t32_sb[:], t32_d[:])
            bias_sb = cst.tile([L, NCH], F32, tag="bias")
            nc.gpsimd.dma_start(bias_sb[:], biasT_d[:])
            wloc_sb, awide_sb, dex_sb = [], [], []
            for c in range(NCH):
                wt = cst.tile([L, L], BF16, tag=f"wloc{c}")
                nc.gpsimd.dma_start(wt[:], wloc_d[c])
                wloc_sb.append(wt)
                at = cst.tile([L, 2 * NCH], BF16, tag=f"awide{c}")
                nc.gpsimd.dma_start(at[:], awide_d[c])
                awide_sb.append(at)
                dt = cst.tile([2 * NCH, L], BF16, tag=f"dex{c}")
                nc.gpsimd.dma_start(dt[:], dex_d[c])
                dex_sb.append(dt)

            # x tiles resident; loads ordered rb-major on sync queue
            xs = [xp.tile([L, R], BF16, tag=f"x{c}", name=f"x{c}")
                  for c in range(NCH)]
            for rb in range(RB):
                for c in range(NCH):
                    nc.sync.dma_start(
                        xs[c][:, RBS * rb:RBS * (rb + 1)],
                        xT_d[L * c:L * (c + 1), RBS * rb:RBS * (rb + 1)])

            def emit_contraction(rb):
                pall = psc.tile([2 * NCH, RBS], F32, tag="pall",
                                name=f"pall{rb}")
                for c in range(NCH):
                    nc.tensor.matmul(
                        pall[:], awide_sb[c][:],
                        xs[c][:, RBS * rb:RBS * (rb + 1)],
                        start=(c == 0), stop=(c == NCH - 1))
                return pall

            def emit_combine(rb, pall):
                pall_sb = dr.tile([2 * NCH, RBS], BF16, tag="pall_sb",
                                  name=f"pallsb{rb}")
                nc.vector.tensor_copy(pall_sb[:], pall[:])
                hps = psh.tile([2 * NCH, RBS], F32, tag="hps",
                               name=f"hps{rb}")
                nc.tensor.matmul(hps[:], t32_sb[:], pall_sb[:],
                                 start=True, stop=True)
                hs = dr.tile([2 * NCH, RBS], BF16, tag="hs", name=f"hs{rb}")
                nc.vector.tensor_copy(hs[:], hps[:])
                return hs

            def emit_main(rb, hs, c):
                po = pop.tile([L, RBS], F32, tag="po", name=f"po{rb}_{c}")
                nc.tensor.matmul(po[:], wloc_sb[c][:],
                                 xs[c][:, RBS * rb:RBS * (rb + 1)],
                                 start=True, stop=False)
                nc.tensor.matmul(po[:], dex_sb[c][:], hs[:],
                                 start=False, stop=True)
                ob = osb.tile([L, RBS], BF16, tag="ob", name=f"ob{rb}_{c}")
                nc.scalar.activation(
                    ob[:], po[:], mybir.ActivationFunctionType.Identity,
                    bias=bias_sb[:, c:c + 1])
                nc.sync.dma_start(
                    outT_d[L * c:L * (c + 1), RBS * rb:RBS * (rb + 1)],
                    ob[:])

            pall0 = emit_contraction(0)
            hs0 = emit_combine(0, pall0)
            # interleave rb1 contractions with rb0 mains
            pall1 = psc.tile([2 * NCH, RBS], F32, tag="pall", name="pall1")
            for c in range(NCH):
                nc.tensor.matmul(
                    pall1[:], awide_sb[c][:], xs[c][:, RBS:],
                    start=(c == 0), stop=(c == NCH - 1))
                emit_main(0, hs0, c)
            hs1 = emit_combine(1, pall1)
            for c in range(NCH):
                emit_main(1, hs1, c)

    nc.compile()
    return nc


def _host_prep(weight, bias, decay_value):
    w0 = weight[0].astype(np.float64)
    w1 = weight[1].astype(np.float64)
    d0 = float(np.clip(np.float32(decay_value[0, 0]), 0.9, 1.0))
    d1 = float(np.clip(np.float32(decay_value[1, 0]), 0.9, 1.0))
    ii = np.arange(L, dtype=np.float64)[:, None]   # local row (s)
    jj = np.arange(L, dtype=np.float64)[None, :]   # local col (t)
    mask = jj >= ii
    pw = np.where(mask, jj - ii, 0.0) / DC

    wloc = np.zeros((NCH, L, L), dtype=NBF16)
    awide = np.zeros((NCH, L, 2 * NCH), dtype=NBF16)
    dex = np.zeros((NCH, 2 * NCH, L), dtype=NBF16)
    j1 = np.arange(L, dtype=np.float64)
    for c in range(NCH):
        w0c = w0[L * c:L * (c + 1)]
        w1c = w1[L * c:L * (c + 1)]
        wl = np.where(mask, w0c[:, None] * d0 ** pw + w1c[None, :] * d1 ** pw,
                      0.0)
        wloc[c] = wl.astype(NBF16)
        awide[c, :, 2 * c] = (w0c * d0 ** ((L - j1) / DC)).astype(NBF16)
        awide[c, :, 2 * c + 1] = (d1 ** ((L - j1) / DC)).astype(NBF16)
        dex[c, 2 * c, :] = (d0 ** (j1 / DC)).astype(NBF16)
        dex[c, 2 * c + 1, :] = (w1c * d1 ** (j1 / DC)).astype(NBF16)

    t32 = np.zeros((2 * NCH, 2 * NCH), dtype=NBF16)
    for c in range(NCH):          # destination chunk
        for cp in range(c):       # source chunk
            k = L * (c - cp - 1) / DC
            t32[2 * cp, 2 * c] = np.float64(d0) ** k
            t32[2 * cp + 1, 2 * c + 1] = np.float64(d1) ** k

    biasT = np.ascontiguousarray(
        bias.astype(np.float32).reshape(NCH, L).T)
    return wloc, awide, dex, t32, biasT


def make_in_maps(x, weight, bias, decay_value):
    wloc, awide, dex, t32, biasT = _host_prep(weight, bias, decay_value)
    x2 = np.asarray(x, dtype=np.float32).reshape(B * E, S)
    in_maps = []
    for c in range(N_CORES):
        xT_c = np.ascontiguousarray(
            x2[R * c:R * (c + 1), :].T).astype(NBF16)
        in_maps.append({
            "xT": xT_c, "wloc": wloc, "awide": awide, "dex": dex,
            "t32": t32, "biasT": biasT,
        })
    return in_maps


def kernel(x, weight, bias, decay_value, index=0, recurrent=0, **_):
    global _PROGRAM
    x = np.asarray(x, dtype=np.float32)
    weight = np.asarray(weight, dtype=np.float32)
    bias = np.asarray(bias, dtype=np.float32)
    decay_value = np.asarray(decay_value, dtype=np.float32)

    if _PROGRAM is None:
        _PROGRAM = _build_program()
    nc = _PROGRAM

    in_maps = make_in_maps(x, weight, bias, decay_value)
    res = run_bass_kernel_spmd(nc, in_maps, core_ids=list(range(N_CORES)))
    out = np.empty((B * E, S), dtype=np.float32)
    for c in range(N_CORES):
        out[R * c:R * (c + 1), :] = res.results[c]["outT"].astype(
            np.float32).T
    return out.reshape(B, E, S)


# revision 6
# speedup vs baseline: 2.5386x; 1.2554x over previous
"""Trainium2 Bass kernel for CombinedRepeatCausalLinear (parallel forward).

Computes out[b,e,t] = sum_s x[b,e,s] * W[s,t] + bias[t] where
  W[s,t] = mask(t>=s) * (w0[s]*d0^(t-s) + w1[t]*d1^(t-s))
for S = 2048, x of shape (8, 1024, 2048) fp32.

Strategy (8 NeuronCores, data-parallel over batch):
  W is rank-2 + causal, so instead of the dense (S,S) matmul we use the
  chunked linear-recurrence form with chunk L=128 (16 chunks):
    out[t, r] = within-chunk triangular part + decayed cross-chunk state.
  Per core (r = 1024 rows, bf16 I/O, fp32 PSUM accumulation):
    - contraction: per chunk c one matmul with stationary Awide_c
      [128 s x 32] (zeros except cols 2c,2c+1) accumulates all chunk
      states into ONE PSUM tile P[32, r] -- the PE itself assembles the
      partition layout (engine partition access must be 32-aligned).
    - combine: one 32x32 matmul with host-precomputed chunk-decay matrix
      T32 turns P into per-chunk start-of-chunk states H[32, r]; H is
      copied into an hs tile whose row 32 is constant 1.0.
    - main: per chunk, within-chunk matmul (K=128, stationary = masked
      128x128 W block) + cross matmul (K=33, stationary Dex_c zero
      except rows 2c,2c+1 and the bias row 32) accumulate in the same
      PSUM tile; bias rides the constant-1 hs row, so PSUM->SBUF drains
      are plain copies alternating between scalar and vector engines.
  DMA instruction issue costs ~600ns each regardless of size, so
  constants are shipped as one large DMA per family (stationaries are
  free-dim slices), x as 16 full-chunk transfers, and outputs split
  across two queues.
"""

import numpy as np
import ml_dtypes

import concourse.bass as bass
import concourse.mybir as mybir
import concourse.tile as tile
from concourse import bacc
from concourse.bass_utils import run_bass_kernel_spmd

F32 = mybir.dt.float32
BF16 = mybir.dt.bfloat16
NBF16 = ml_dtypes.bfloat16

B = 8
E = 1024
S = 2048
DC = 1.0
N_CORES = 8
R = (B * E) // N_CORES      # rows per core = 1024
L = 128                     # chunk length along s/t
NCH = S // L                # 16 chunks
NST = 2 * NCH               # 32 state rows
RB = 2                      # r blocks
RBS = R // RB               # 512

_PROGRAM = None


def _build_program():
    nc = bacc.Bacc("TRN2", target_bir_lowering=False, debug=False,
                   num_devices=N_CORES)

    xT_d = nc.declare_dram_parameter("xT", [S, R], BF16, isOutput=False)
    wbig_d = nc.declare_dram_parameter("wbig", [L, NCH * L], BF16,
                                       isOutput=False)
    awbig_d = nc.declare_dram_parameter("awbig", [L, NCH * NST], BF16,
                                        isOutput=False)
    dexbig_d = nc.declare_dram_parameter("dexbig", [NST + 1, NCH * L], BF16,
                                         isOutput=False)
    t32_d = nc.declare_dram_parameter("t32", [NST, NST], BF16,
                                      isOutput=False)
    outT_d = nc.declare_dram_parameter("outT", [S, R], BF16, isOutput=True)

    with tile.TileContext(nc) as tc:
        with (
            tc.tile_pool(name="cst", bufs=1) as cst,
            tc.tile_pool(name="xp", bufs=1) as xp,
            tc.tile_pool(name="dr", bufs=4) as dr,
            tc.tile_pool(name="osb", bufs=8) as osb,
            tc.tile_pool(name="psc", bufs=2, space="PSUM") as psc,
            tc.tile_pool(name="psh", bufs=1, space="PSUM") as psh,
            tc.tile_pool(name="pop", bufs=5, space="PSUM") as pop,
        ):
            # resident constants, one DMA each (gpsimd queue)
            awbig = cst.tile([L, NCH * NST], BF16, tag="awbig")
            nc.gpsimd.dma_start(awbig[:], awbig_d[:])
            t32_sb = cst.tile([NST, NST], BF16, tag="t32")
            nc.gpsimd.dma_start(t32_sb[:], t32_d[:])
            wbig = cst.tile([L, NCH * L], BF16, tag="wbig")
            nc.gpsimd.dma_start(wbig[:], wbig_d[:])
            dexbig = cst.tile([NST + 1, NCH * L], BF16, tag="dexbig")
            nc.gpsimd.dma_start(dexbig[:], dexbig_d[:])

            # x tiles resident; one full-chunk DMA each (sync queue)
            xs = [xp.tile([L, R], BF16, tag=f"x{c}", name=f"x{c}")
                  for c in range(NCH)]
            for c in range(NCH):
                nc.sync.dma_start(xs[c][:], xT_d[L * c:L * (c + 1), :])

            # contractions: accumulate chunk states into P[32, r] per rb
            palls = []
            for rb in range(RB):
                palls.append(psc.tile([NST, RBS], F32, tag="pall",
                                      name=f"pall{rb}"))
            for c in range(NCH):
                aw = awbig[:, NST * c:NST * (c + 1)]
                for rb in range(RB):
                    nc.tensor.matmul(
                        palls[rb][:], aw, xs[c][:, RBS * rb:RBS * (rb + 1)],
                        start=(c == 0), stop=(c == NCH - 1))

            # combine: H = T32.T @ P; hs row 32 stays the memset 1.0
            hss = []
            for rb in range(RB):
                pall_sb = dr.tile([NST, RBS], BF16, tag="pall_sb",
                                  name=f"pallsb{rb}")
                nc.vector.tensor_copy(pall_sb[:], palls[rb][:])
                hps = psh.tile([NST, RBS], F32, tag="hps", name=f"hps{rb}")
                nc.tensor.matmul(hps[:], t32_sb[:], pall_sb[:],
                                 start=True, stop=True)
                hs = dr.tile([NST + 1, RBS], BF16, tag="hs", name=f"hs{rb}")
                nc.gpsimd.memset(hs[:], 1.0)
                nc.vector.tensor_copy(hs[0:NST, :], hps[:])
                hss.append(hs)

            # mains: within (K=128) + cross-with-bias (K=33) per (c, rb)
            for c in range(NCH):
                wl = wbig[:, L * c:L * (c + 1)]
                dx = dexbig[:, L * c:L * (c + 1)]
                for rb in range(RB):
                    po = pop.tile([L, RBS], F32, tag="po",
                                  name=f"po{rb}_{c}")
                    nc.tensor.matmul(po[:], wl,
                                     xs[c][:, RBS * rb:RBS * (rb + 1)],
                                     start=True, stop=False)
                    nc.tensor.matmul(po[:], dx, hss[rb][:],
                                     start=False, stop=True)
                    ob = osb.tile([L, RBS], BF16, tag="ob",
                                  name=f"ob{rb}_{c}")
                    if (c + rb) % 2 == 0:
                        nc.scalar.activation(
                            ob[:], po[:],
                            mybir.ActivationFunctionType.Identity)
                    else:
                        nc.vector.tensor_copy(ob[:], po[:])
                    q = nc.gpsimd if rb == 0 else nc.sync
                    q.dma_start(
                        outT_d[L * c:L * (c + 1), RBS * rb:RBS * (rb + 1)],
                        ob[:])

    nc.compile()
    return nc


def _host_prep(weight, bias, decay_value):
    w0 = weight[0].astype(np.float64)
    w1 = weight[1].astype(np.float64)
    d0 = float(np.clip(np.float32(decay_value[0, 0]), 0.9, 1.0))
    d1 = float(np.clip(np.float32(decay_value[1, 0]), 0.9, 1.0))
    ii = np.arange(L, dtype=np.float64)[:, None]   # local row (s)
    jj = np.arange(L, dtype=np.float64)[None, :]   # local col (t)
    mask = jj >= ii
    pw = np.where(mask, jj - ii, 0.0) / DC
    j1 = np.arange(L, dtype=np.float64)

    wbig = np.zeros((L, NCH * L), dtype=NBF16)
    awbig = np.zeros((L, NCH * NST), dtype=NBF16)
    dexbig = np.zeros((NST + 1, NCH * L), dtype=NBF16)
    for c in range(NCH):
        w0c = w0[L * c:L * (c + 1)]
        w1c = w1[L * c:L * (c + 1)]
        wl = np.where(mask, w0c[:, None] * d0 ** pw + w1c[None, :] * d1 ** pw,
                      0.0)
        wbig[:, L * c:L * (c + 1)] = wl.astype(NBF16)
        awbig[:, NST * c + 2 * c] = (w0c * d0 ** ((L - j1) / DC)
                                     ).astype(NBF16)
        awbig[:, NST * c + 2 * c + 1] = (d1 ** ((L - j1) / DC)
                                         ).astype(NBF16)
        dexbig[2 * c, L * c:L * (c + 1)] = (d0 ** (j1 / DC)).astype(NBF16)
        dexbig[2 * c + 1, L * c:L * (c + 1)] = (w1c * d1 ** (j1 / DC)
                                                ).astype(NBF16)
        dexbig[NST, L * c:L * (c + 1)] = bias[L * c:L * (c + 1)].astype(
            NBF16)

    t32 = np.zeros((NST, NST), dtype=NBF16)
    for c in range(NCH):          # destination chunk
        for cp in range(c):       # source chunk
            k = L * (c - cp - 1) / DC
            t32[2 * cp, 2 * c] = np.float64(d0) ** k
            t32[2 * cp + 1, 2 * c + 1] = np.float64(d1) ** k

    return wbig, awbig, dexbig, t32


def make_in_maps(x, weight, bias, decay_value):
    wbig, awbig, dexbig, t32 = _host_prep(weight, bias, decay_value)
    x2 = np.asarray(x, dtype=np.float32).reshape(B * E, S)
    in_maps = []
    for c in range(N_CORES):
        xT_c = np.ascontiguousarray(
            x2[R * c:R * (c + 1), :].T).astype(NBF16)
        in_maps.append({
            "xT": xT_c, "wbig": wbig, "awbig": awbig, "dexbig": dexbig,
            "t32": t32,
        })
    return in_maps


def kernel(x, weight, bias, decay_value, index=0, recurrent=0, **_):
    global _PROGRAM
    x = np.asarray(x, dtype=np.float32)
    weight = np.asarray(weight, dtype=np.float32)
    bias = np.asarray(bias, dtype=np.float32)
    decay_value = np.asarray(decay_value, dtype=np.float32)

    if _PROGRAM is None:
        _PROGRAM = _build_program()
    nc = _PROGRAM

    in_maps = make_in_maps(x, weight, bias, decay_value)
    res = run_bass_kernel_spmd(nc, in_maps, core_ids=list(range(N_CORES)))
    out = np.empty((B * E, S), dtype=np.float32)
    for c in range(N_CORES):
        out[R * c:R * (c + 1), :] = res.results[c]["outT"].astype(
            np.float32).T
    return out.reshape(B, E, S)
